# revision 3
# baseline (speedup 1.0000x reference)
"""Trainium2 Bass kernel for an LSTM pointer-network decoder.

Problem shapes: B=64, L=512, E=256, H=256.
reference computes, per step t in 0..L-1:
    gates = x @ W_ih.T + b_ih + h @ W_hh.T + b_hh          (gate order i,f,g,o)
    c = sigmoid(f)*c + sigmoid(i)*tanh(g);  h = sigmoid(o)*tanh(c)
    inp = h @ W_in.T + b_in
    att[b,l] = sum_h V[h] * tanh(inp[b,h] + ctx_proj[b,l,h])
    out[t] = where(l < t, -inf, att)        # mask zeroes position t after step t
    x_next = embedded_inputs[:, t]          # pointer index forced to t
Returns (outputs[B,L,L], pointers[B,L] int32, hT[B,H], cT[B,H]).

Key structure: the attention output never feeds the recurrence, and the input
sequence is known up front, so the LSTM input projections are precomputed as
one batched matmul, the recurrence runs with per-step matmuls, and the O(B*L*L*H)
attention is a throughput sweep (DVE broadcast-add -> ACT tanh -> PE V-reduce
with a sliding-window V so 128 t-rows share one PSUM tile).

Sharding: data-parallel over batch, 8 NeuronCores, B_loc = 8 per core.
"""

import numpy as np
import ml_dtypes

B, L, E, H = 64, 512, 256, 256
NCORES = 8
BL = B // NCORES          # 8 batches per core
H4 = 4 * H                # 1024
NTB = L // 128            # 4 t-blocks of 128
NBLK = L // 64            # 8 inp-projection blocks of 64 steps
NEG_INF = np.float32(-np.inf)

_PROG = None  # cached (nc, meta)


def _build_program():
    import concourse.mybir as mybir
    import concourse.tile as tile
    from concourse import bacc

    dt = mybir.dt
    f32, bf16 = dt.float32, dt.bfloat16
    AFT = mybir.ActivationFunctionType
    Alu = mybir.AluOpType

    nc = bacc.Bacc("TRN2", target_bir_lowering=False, debug=False)

    # ---- DRAM I/O (all host-prepped, see kernel()) ----
    d_xt = nc.dram_tensor("xt", [257, L * BL], bf16, kind="ExternalInput")      # [E; ones] x (t,b)
    d_ctxt = nc.dram_tensor("ctxt", [256, BL * L], bf16, kind="ExternalInput")  # h_in x (b,l)
    d_wiht = nc.dram_tensor("wiht", [257, H4], bf16, kind="ExternalInput")      # [W_ih.T; beta_g], gates [i,f,o,g]
    d_whht = nc.dram_tensor("whht", [256, H4], bf16, kind="ExternalInput")      # W_hh.T, gates [i,f,o,g]
    d_wint = nc.dram_tensor("wint", [256, H], bf16, kind="ExternalInput")       # W_in.T
    d_wctxt = nc.dram_tensor("wctxt", [256, H], bf16, kind="ExternalInput")     # W_ctx.T
    d_beta2 = nc.dram_tensor("beta2", [256, 1], f32, kind="ExternalInput")      # b_ctx + b_in
    d_vwin = nc.dram_tensor("vwin", [256, 257], bf16, kind="ExternalInput")     # V at col 128, else 0
    d_mtri = nc.dram_tensor("mtri", [128, 512], f32, kind="ExternalInput")      # -inf strict lower tri
    d_i8f = nc.dram_tensor("i8f", [8, 8], f32, kind="ExternalInput")
    d_i8b = nc.dram_tensor("i8b", [8, 8], bf16, kind="ExternalInput")
    d_h0t = nc.dram_tensor("h0t", [256, BL], bf16, kind="ExternalInput")        # h0 transposed
    d_c0 = nc.dram_tensor("c0", [BL, H], f32, kind="ExternalInput")

    d_att = nc.dram_tensor("att", [BL, L, L], f32, kind="ExternalOutput")
    d_hT = nc.dram_tensor("hT_out", [BL, H], f32, kind="ExternalOutput")
    d_cT = nc.dram_tensor("cT_out", [BL, H], f32, kind="ExternalOutput")
    d_xp = nc.dram_tensor("xp_scratch", [L * BL, H4], bf16)                     # internal

    att_ap = d_att.ap()
    xp_ap = d_xp.ap()

    with tile.TileContext(nc) as tc:
        # ---------- persistent SBUF residents ----------
        with tc.tile_pool(name="resident", bufs=1) as rp:
            whht_sb = [rp.tile([128, H4], bf16, tag=f"whht{k}", name=f"whht{k}") for k in range(2)]
            wint_sb = [rp.tile([128, H], bf16, tag=f"wint{k}", name=f"wint{k}") for k in range(2)]
            vwin_sb = [rp.tile([128, 257], bf16, tag=f"vwin{k}", name=f"vwin{k}") for k in range(2)]
            mtri_sb = rp.tile([128, 512], f32, tag="mtri")
            i8f_sb = rp.tile([8, 8], f32, tag="i8f")
            i8b_sb = rp.tile([8, 8], bf16, tag="i8b")
            h0t_sb = [rp.tile([128, BL], bf16, tag=f"h0t{k}", name=f"h0t{k}") for k in range(2)]
            beta2_sb = [rp.tile([128, 1], f32, tag=f"beta2{k}", name=f"beta2{k}") for k in range(2)]
            inf_sb = rp.tile([128, 384], f32, tag="infc")
            cpT_sb = [rp.tile([128, BL * L], bf16, tag=f"cpT{k}", name=f"cpT{k}") for k in range(2)]
            inpT_sb = [rp.tile([128, L * BL], f32, tag=f"inpT{k}", name=f"inpT{k}") for k in range(2)]
            # persistent recurrence state
            h_sb = rp.tile([BL, H], f32, tag="h_state")
            c_sb = rp.tile([BL, H], f32, tag="c_state")

            for k in range(2):
                nc.sync.dma_start(whht_sb[k][:], d_whht.ap()[k * 128:(k + 1) * 128, :])
                nc.sync.dma_start(wint_sb[k][:], d_wint.ap()[k * 128:(k + 1) * 128, :])
                nc.sync.dma_start(vwin_sb[k][:], d_vwin.ap()[k * 128:(k + 1) * 128, :])
                nc.sync.dma_start(h0t_sb[k][:], d_h0t.ap()[k * 128:(k + 1) * 128, :])
                nc.sync.dma_start(beta2_sb[k][:], d_beta2.ap()[k * 128:(k + 1) * 128, :])
            nc.sync.dma_start(mtri_sb[:], d_mtri.ap()[:])
            nc.sync.dma_start(i8f_sb[:], d_i8f.ap()[:])
            nc.sync.dma_start(i8b_sb[:], d_i8b.ap()[:])
            nc.sync.dma_start(c_sb[:], d_c0.ap()[:])
            nc.gpsimd.memset(inf_sb[:], float("-inf"))

            # ---------- phase A: x_proj = [X;1] @ [W_ih.T; beta] -> xp_scratch ----
            with tc.tile_pool(name="pha", bufs=2) as pa, \
                 tc.tile_pool(name="pha_ps", bufs=2, space="PSUM") as pa_ps, \
                 tc.tile_pool(name="pha_out", bufs=3) as pa_out:
                xt_sb = [pa.tile([128, L * BL], bf16, tag=f"xt{k}", name=f"xt{k}") for k in range(2)]
                xt1_sb = pa.tile([1, L * BL], bf16, tag="xt2")
                wih_sb = [pa.tile([128, H4], bf16, tag=f"wih{k}", name=f"wih{k}") for k in range(2)]
                wih1_sb = pa.tile([1, H4], bf16, tag="wih2")
                for k in range(2):
                    nc.sync.dma_start(xt_sb[k][:], d_xt.ap()[k * 128:(k + 1) * 128, :])
                    nc.sync.dma_start(wih_sb[k][:], d_wiht.ap()[k * 128:(k + 1) * 128, :])
                nc.sync.dma_start(xt1_sb[:], d_xt.ap()[256:257, :])
                nc.sync.dma_start(wih1_sb[:], d_wiht.ap()[256:257, :])
                for m in range(32):            # (t,b) tiles of 128 rows
                    for nh in range(2):        # 1024 = 2 x 512
                        ps = pa_ps.tile([128, 512], f32, tag="xp_ps")
                        nsl = slice(nh * 512, (nh + 1) * 512)
                        msl = slice(m * 128, (m + 1) * 128)
                        nc.tensor.matmul(ps[:], xt_sb[0][:, msl], wih_sb[0][:, nsl],
                                         start=True, stop=False)
                        nc.tensor.matmul(ps[:], xt_sb[1][:, msl], wih_sb[1][:, nsl],
                                         start=False, stop=False)
                        nc.tensor.matmul(ps[:], xt1_sb[:, msl], wih1_sb[:, nsl],
                                         start=False, stop=True)
                        ot = pa_out.tile([128, 512], bf16, tag="xp_out")
                        nc.vector.tensor_copy(ot[:], ps[:])
                        nc.sync.dma_start(xp_ap[msl, nsl], ot[:])

            # ---------- phase B: ctx_proj^T = W_ctx @ ctx^T + (b_ctx+b_in) ------
            with tc.tile_pool(name="phb", bufs=1) as pb, \
                 tc.tile_pool(name="phb_ps", bufs=2, space="PSUM") as pb_ps:
                ctxt_sb = [pb.tile([128, BL * L], bf16, tag=f"ctxt{k}", name=f"ctxt{k}") for k in range(2)]
                wctx_sb = [pb.tile([128, H], bf16, tag=f"wctx{k}", name=f"wctx{k}") for k in range(2)]
                for k in range(2):
                    nc.sync.dma_start(ctxt_sb[k][:], d_ctxt.ap()[k * 128:(k + 1) * 128, :])
                    nc.sync.dma_start(wctx_sb[k][:], d_wctxt.ap()[k * 128:(k + 1) * 128, :])
                for m in range(2):             # h_out tile
                    for nchunk in range(8):    # BL*L = 4096 = 8 x 512
                        ps = pb_ps.tile([128, 512], f32, tag="cp_ps")
                        nsl = slice(nchunk * 512, (nchunk + 1) * 512)
                        for k in range(2):
                            nc.tensor.matmul(ps[:], wctx_sb[k][:, m * 128:(m + 1) * 128],
                                             ctxt_sb[k][:, nsl],
                                             start=(k == 0), stop=(k == 1))
                        nc.vector.tensor_scalar_add(cpT_sb[m][:, nsl], ps[:],
                                                    beta2_sb[m][:, 0:1])

            # ---------- main: recurrence + attention ----------
            with tc.tile_pool(name="gates_ps", bufs=1, space="PSUM") as gps_pool, \
                 tc.tile_pool(name="tr_ps", bufs=1, space="PSUM") as trp_pool, \
                 tc.tile_pool(name="ps512", bufs=5, space="PSUM") as ps512, \
                 tc.tile_pool(name="xp_in", bufs=6) as xp_pool, \
                 tc.tile_pool(name="hist", bufs=2) as hist_pool, \
                 tc.tile_pool(name="gtmp", bufs=2) as gtmp, \
                 tc.tile_pool(name="targ", bufs=3) as targ_pool, \
                 tc.tile_pool(name="tanh", bufs=6) as tanh_pool, \
                 tc.tile_pool(name="attev", bufs=3) as attev:

                hist_cur = None
                hist_prev = None
                att_done = 0

                def emit_inp_block(blk, hist_tiles):
                    # inp^T[:, blk*512:(blk+1)*512] = W_in @ h_hist (no bias)
                    for m in range(2):
                        ps = ps512.tile([128, 512], f32, tag="att_ps", name="att_ps")
                        for k in range(2):
                            nc.tensor.matmul(ps[:], wint_sb[k][:, m * 128:(m + 1) * 128],
                                             hist_tiles[k][:],
                                             start=(k == 0), stop=(k == 1))
                        nc.vector.tensor_copy(
                            inpT_sb[m][:, blk * 512:(blk + 1) * 512], ps[:])

                def emit_attention(TB):
                    Nw = 512 - 128 * TB
                    l0 = 128 * TB
                    for bg in range(4):            # 2 batches per group
                        bpair = (2 * bg, 2 * bg + 1)
                        att_ps = {}
                        for bi, b in enumerate(bpair):
                            att_ps[b] = ps512.tile([128, 512], f32, tag="att_ps", name="att_ps")
                        tanh_tiles = {}
                        for tb8 in range(TB * 16, (TB + 1) * 16):
                            # build tanh(inp + cp) tiles for 8 t's x 2 k x 2 b
                            for b in bpair:
                                for k in range(2):
                                    tg = targ_pool.tile([128, 8 * 512], bf16, tag="targ")
                                    for i in range(8):
                                        t = tb8 * 8 + i
                                        nc.vector.tensor_scalar_add(
                                            tg[:, i * Nw:(i + 1) * Nw],
                                            cpT_sb[k][:, b * 512 + l0: (b + 1) * 512],
                                            inpT_sb[k][:, t * 8 + b: t * 8 + b + 1])
                                    th = tanh_pool.tile([128, 8 * 512], bf16, tag="tanh")
                                    nc.scalar.activation(th[:, 0:8 * Nw], tg[:, 0:8 * Nw],
                                                         AFT.Tanh)
                                    tanh_tiles[(k, b)] = th
                            for i in range(8):
                                t = tb8 * 8 + i
                                j = t - TB * 128
                                for k in range(2):
                                    lw = vwin_sb[k][:, 128 - j:256 - j]
                                    for b in bpair:
                                        nc.tensor.matmul(
                                            att_ps[b][:, 0:Nw], lw,
                                            tanh_tiles[(k, b)][:, i * Nw:(i + 1) * Nw],
                                            start=(j == 0 and k == 0),
                                            stop=(j == 127 and k == 1),
                                            skip_group_check=True)
                        for b in bpair:
                            ev = attev.tile([128, 512], f32, tag="attev")
                            nc.vector.tensor_add(ev[:, 0:Nw], att_ps[b][:, 0:Nw],
                                                 mtri_sb[:, 0:Nw])
                            nc.sync.dma_start(
                                att_ap[b, TB * 128:(TB + 1) * 128, l0:512], ev[:, 0:Nw])
                            if TB > 0:
                                nc.sync.dma_start(
                                    att_ap[b, TB * 128:(TB + 1) * 128, 0:l0],
                                    inf_sb[:, 0:l0])

                for t in range(L):
                    blk = t // 64
                    if t % 64 == 0:
                        hist_prev_tiles = hist_cur
                        hist_cur = [hist_pool.tile([128, 64 * BL], bf16, tag=f"hist{k}", name=f"hist{k}")
                                    for k in range(2)]
                    # ---- gates = x_proj[t] + h @ W_hh.T  (PSUM accumulate) ----
                    gps = gps_pool.tile([BL, H4], f32, tag="gates")
                    xpt = xp_pool.tile([BL, H4], bf16, tag="xp")
                    nc.sync.dma_start(xpt[:], xp_ap[t * BL:(t + 1) * BL, :])
                    for nh in range(2):
                        nsl = slice(nh * 512, (nh + 1) * 512)
                        nc.tensor.matmul(gps[:, nsl], i8b_sb[:], xpt[:, nsl],
                                         start=True, stop=False)
                        for k in range(2):
                            if t == 0:
                                lhsT = h0t_sb[k][:]
                            else:
                                src = hist_cur if (t % 64) != 0 else hist_prev_tiles
                                c0_ = ((t - 1) % 64) * BL
                                lhsT = src[k][:, c0_:c0_ + BL]
                            nc.tensor.matmul(gps[:, nsl], lhsT, whht_sb[k][:, nsl],
                                             start=False, stop=(k == 1))
                    # ---- nonlinearities ----
                    sifo = gtmp.tile([BL, 768], f32, tag="sifo")
                    tg_t = gtmp.tile([BL, 256], f32, tag="tg")
                    nc.scalar.activation(sifo[:], gps[:, 0:768], AFT.Sigmoid)
                    nc.scalar.activation(tg_t[:], gps[:, 768:1024], AFT.Tanh)
                    # ---- c update ----
                    t1 = gtmp.tile([BL, 256], f32, tag="t1")
                    t2 = gtmp.tile([BL, 256], f32, tag="t2")
                    nc.vector.tensor_mul(t1[:], sifo[:, 256:512], c_sb[:])
                    nc.vector.tensor_mul(t2[:], sifo[:, 0:256], tg_t[:])
                    nc.vector.tensor_add(c_sb[:], t1[:], t2[:])
                    tc_t = gtmp.tile([BL, 256], f32, tag="tc")
                    nc.scalar.activation(tc_t[:], c_sb[:], AFT.Tanh)
                    nc.vector.tensor_mul(h_sb[:], sifo[:, 512:768], tc_t[:])
                    # ---- h^T into history (PE transpose) ----
                    trp = trp_pool.tile([128, 16], f32, tag="tr")
                    nc.tensor.transpose(trp[:, 0:8], h_sb[:, 0:128], i8f_sb[:])
                    nc.tensor.transpose(trp[:, 8:16], h_sb[:, 128:256], i8f_sb[:])
                    c8 = (t % 64) * BL
                    nc.vector.tensor_copy(hist_cur[0][:, c8:c8 + BL], trp[:, 0:8])
                    nc.vector.tensor_copy(hist_cur[1][:, c8:c8 + BL], trp[:, 8:16])

                    if t % 64 == 63:
                        emit_inp_block(blk, hist_cur)
                    if t % 128 == 127:
                        emit_attention(t // 128)

                nc.sync.dma_start(d_hT.ap()[:], h_sb[:])
                nc.sync.dma_start(d_cT.ap()[:], c_sb[:])

    nc.compile()
    return nc


def _get_program():
    global _PROG
    if _PROG is None:
        _PROG = _build_program()
    return _PROG


def _prep_core_inputs(inputs, core):
    """Host-side layout prep for one core's batch shard (glue only)."""
    bf16 = ml_dtypes.bfloat16
    bsl = slice(core * BL, (core + 1) * BL)
    emb = np.asarray(inputs["embedded_inputs"])[bsl]    # [BL, L, E]
    dec = np.asarray(inputs["decoder_input"])[bsl]      # [BL, E]
    h0 = np.asarray(inputs["h0"])[bsl]
    c0 = np.asarray(inputs["c0"])[bsl]
    ctx = np.asarray(inputs["context"])[bsl]            # [BL, L, H]

    gate_perm = np.r_[0:256, 256:512, 768:1024, 512:768]  # [i,f,g,o] -> [i,f,o,g]
    W_ih = np.asarray(inputs["W_ih"])[gate_perm]
    W_hh = np.asarray(inputs["W_hh"])[gate_perm]
    beta_g = (np.asarray(inputs["b_ih"]) + np.asarray(inputs["b_hh"]))[gate_perm]
    W_in = np.asarray(inputs["W_in"])
    W_ctx = np.asarray(inputs["W_ctx"])
    beta2 = (np.asarray(inputs["b_in"]) + np.asarray(inputs["b_ctx"]))
    V = np.asarray(inputs["V"])

    # X[t] = dec if t==0 else emb[:, t-1];  xt = [X^T; ones], cols (t, b)
    X = np.concatenate([dec[:, None, :], emb[:, :L - 1, :]], axis=1)  # [BL, L, E]
    xt = np.empty((257, L * BL), dtype=bf16)
    xt[:256] = X.transpose(2, 1, 0).reshape(E, L * BL).astype(bf16)
    xt[256] = np.ones(L * BL, dtype=bf16)

    ctxt = ctx.transpose(2, 0, 1).reshape(H, BL * L).astype(bf16)     # (h, (b,l))

    wiht = np.empty((257, H4), dtype=bf16)
    wiht[:256] = W_ih.T.astype(bf16)
    wiht[256] = beta_g.astype(bf16)

    vwin = np.zeros((256, 257), dtype=bf16)
    vwin[:, 128] = V.astype(bf16)

    mtri = np.zeros((128, 512), dtype=np.float32)
    r = np.arange(128)
    mask = np.arange(512)[None, :] < r[:, None]
    mtri[mask] = NEG_INF

    return {
        "xt": xt,
        "ctxt": ctxt,
        "wiht": wiht,
        "whht": np.ascontiguousarray(W_hh.T).astype(bf16),
        "wint": np.ascontiguousarray(W_in.T).astype(bf16),
        "wctxt": np.ascontiguousarray(W_ctx.T).astype(bf16),
        "beta2": beta2.astype(np.float32).reshape(256, 1),
        "vwin": vwin,
        "mtri": mtri,
        "i8f": np.eye(8, dtype=np.float32),
        "i8b": np.eye(8, dtype=bf16),
        "h0t": np.ascontiguousarray(h0.T).astype(bf16),
        "c0": np.ascontiguousarray(c0).astype(np.float32),
    }


LAST_RESULT = None


def kernel(**inputs):
    from concourse.bass_utils import run_bass_kernel_spmd

    nc = _get_program()
    in_maps = [_prep_core_inputs(inputs, c) for c in range(NCORES)]
    res = run_bass_kernel_spmd(nc, in_maps, list(range(NCORES)))
    global LAST_RESULT
    LAST_RESULT = res

    outputs = np.empty((B, L, L), dtype=np.float32)
    hT = np.empty((B, H), dtype=np.float32)
    cT = np.empty((B, H), dtype=np.float32)
    for c in range(NCORES):
        bsl = slice(c * BL, (c + 1) * BL)
        outputs[bsl] = res.results[c]["att"]
        hT[bsl] = res.results[c]["hT_out"]
        cT[bsl] = res.results[c]["cT_out"]
    pointers = np.broadcast_to(np.arange(L, dtype=np.int32), (B, L)).copy()
    return outputs, pointers, hT, cT


# revision 9
# speedup vs baseline: 1.1498x; 1.1498x over previous
"""Trainium2 Bass kernel for an LSTM pointer-network decoder.

Problem shapes: B=64, L=512, E=256, H=256.
reference computes, per step t in 0..L-1:
    gates = x @ W_ih.T + b_ih + h @ W_hh.T + b_hh          (gate order i,f,g,o)
    c = sigmoid(f)*c + sigmoid(i)*tanh(g);  h = sigmoid(o)*tanh(c)
    inp = h @ W_in.T + b_in
    att[b,l] = sum_h V[h] * tanh(inp[b,h] + ctx_proj[b,l,h])
    out[t] = where(l < t, -inf, att)        # mask zeroes position t after step t
    x_next = embedded_inputs[:, t]          # pointer index forced to t
Returns (outputs[B,L,L], pointers[B,L] int32, hT[B,H], cT[B,H]).

Key structure: the attention output never feeds the recurrence, and the input
sequence is known up front, so the LSTM input projections are precomputed as
one batched matmul, the recurrence runs with per-step matmuls, and the O(B*L*L*H)
attention is a throughput sweep (DVE broadcast-add -> ACT tanh -> PE V-reduce
with a sliding-window V so 128 t-rows share one PSUM tile).

Sharding: data-parallel over batch, 8 NeuronCores, B_loc = 8 per core.
"""

import numpy as np
import ml_dtypes

B, L, E, H = 64, 512, 256, 256
NCORES = 8
BL = B // NCORES          # 8 batches per core
H4 = 4 * H                # 1024
NTB = L // 128            # 4 t-blocks of 128
NBLK = L // 64            # 8 inp-projection blocks of 64 steps
NEG_INF = np.float32(-np.inf)

_PROG = None  # cached (nc, meta)


def _build_program():
    import concourse.mybir as mybir
    import concourse.tile as tile
    from concourse import bacc

    dt = mybir.dt
    f32, bf16 = dt.float32, dt.bfloat16
    AFT = mybir.ActivationFunctionType
    Alu = mybir.AluOpType

    nc = bacc.Bacc("TRN2", target_bir_lowering=False, debug=False)

    # ---- DRAM I/O (all host-prepped, see kernel()) ----
    d_xt = nc.dram_tensor("xt", [257, L * BL], bf16, kind="ExternalInput")      # [E; ones] x (t,b)
    d_ctxt = nc.dram_tensor("ctxt", [256, BL * L], bf16, kind="ExternalInput")  # h_in x (b,l)
    d_wiht = nc.dram_tensor("wiht", [257, H4], bf16, kind="ExternalInput")      # [W_ih.T; beta_g], gates [i,f,o,g]
    d_whht = nc.dram_tensor("whht", [256, H4], bf16, kind="ExternalInput")      # W_hh.T, gates [i,f,o,g]
    d_wint = nc.dram_tensor("wint", [256, H], bf16, kind="ExternalInput")       # W_in.T
    d_wctxt = nc.dram_tensor("wctxt", [256, H], bf16, kind="ExternalInput")     # W_ctx.T
    d_beta2 = nc.dram_tensor("beta2", [256, 1], f32, kind="ExternalInput")      # b_ctx + b_in
    d_vwin = nc.dram_tensor("vwin", [256, 257], bf16, kind="ExternalInput")     # V at col 128, else 0
    d_mtri = nc.dram_tensor("mtri", [128, 512], f32, kind="ExternalInput")      # -inf strict lower tri
    d_i8f = nc.dram_tensor("i8f", [8, 8], f32, kind="ExternalInput")
    d_i8b = nc.dram_tensor("i8b", [8, 8], bf16, kind="ExternalInput")
    d_h0t = nc.dram_tensor("h0t", [256, BL], bf16, kind="ExternalInput")        # h0 transposed
    d_c0 = nc.dram_tensor("c0", [BL, H], bf16, kind="ExternalInput")

    d_att = nc.dram_tensor("att", [BL, L, L], f32, kind="ExternalOutput")
    d_hT = nc.dram_tensor("hT_out", [BL, H], f32, kind="ExternalOutput")
    d_cT = nc.dram_tensor("cT_out", [BL, H], f32, kind="ExternalOutput")
    d_xp = nc.dram_tensor("xp_scratch", [L * BL, H4], bf16)                     # internal

    att_ap = d_att.ap()
    xp_ap = d_xp.ap()

    with tile.TileContext(nc) as tc:
        # ---------- persistent SBUF residents ----------
        with tc.tile_pool(name="resident", bufs=1) as rp:
            whht_sb = [rp.tile([128, H4], bf16, tag=f"whht{k}", name=f"whht{k}") for k in range(2)]
            wint_sb = [rp.tile([128, H], bf16, tag=f"wint{k}", name=f"wint{k}") for k in range(2)]
            vwin_sb = [rp.tile([128, 257], bf16, tag=f"vwin{k}", name=f"vwin{k}") for k in range(2)]
            mtri_sb = rp.tile([128, 512], f32, tag="mtri")
            i8f_sb = rp.tile([8, 8], f32, tag="i8f")
            i8b_sb = rp.tile([8, 8], bf16, tag="i8b")
            h0t_sb = [rp.tile([128, BL], bf16, tag=f"h0t{k}", name=f"h0t{k}") for k in range(2)]
            beta2_sb = [rp.tile([128, 1], f32, tag=f"beta2{k}", name=f"beta2{k}") for k in range(2)]
            inf_sb = rp.tile([128, 384], f32, tag="infc")
            cpT_sb = [rp.tile([128, BL * L], bf16, tag=f"cpT{k}", name=f"cpT{k}") for k in range(2)]
            inpT_sb = [rp.tile([128, L * BL], f32, tag=f"inpT{k}", name=f"inpT{k}") for k in range(2)]
            # persistent recurrence state
            h_sb = rp.tile([BL, H], bf16, tag="h_state")
            c_sb = rp.tile([BL, H], bf16, tag="c_state")

            for k in range(2):
                nc.sync.dma_start(whht_sb[k][:], d_whht.ap()[k * 128:(k + 1) * 128, :])
                nc.sync.dma_start(wint_sb[k][:], d_wint.ap()[k * 128:(k + 1) * 128, :])
                nc.sync.dma_start(vwin_sb[k][:], d_vwin.ap()[k * 128:(k + 1) * 128, :])
                nc.sync.dma_start(h0t_sb[k][:], d_h0t.ap()[k * 128:(k + 1) * 128, :])
                nc.sync.dma_start(beta2_sb[k][:], d_beta2.ap()[k * 128:(k + 1) * 128, :])
            nc.sync.dma_start(mtri_sb[:], d_mtri.ap()[:])
            nc.sync.dma_start(i8f_sb[:], d_i8f.ap()[:])
            nc.sync.dma_start(i8b_sb[:], d_i8b.ap()[:])
            nc.sync.dma_start(c_sb[:], d_c0.ap()[:])
            nc.gpsimd.memset(inf_sb[:], float("-inf"))

            # ---------- phase A: x_proj = [X;1] @ [W_ih.T; beta] -> xp_scratch ----
            with tc.tile_pool(name="pha", bufs=2) as pa, \
                 tc.tile_pool(name="pha_ps", bufs=2, space="PSUM") as pa_ps, \
                 tc.tile_pool(name="pha_out", bufs=3) as pa_out:
                xt_sb = [pa.tile([128, L * BL], bf16, tag=f"xt{k}", name=f"xt{k}") for k in range(2)]
                xt1_sb = pa.tile([1, L * BL], bf16, tag="xt2")
                wih_sb = [pa.tile([128, H4], bf16, tag=f"wih{k}", name=f"wih{k}") for k in range(2)]
                wih1_sb = pa.tile([1, H4], bf16, tag="wih2")
                for k in range(2):
                    nc.sync.dma_start(xt_sb[k][:], d_xt.ap()[k * 128:(k + 1) * 128, :])
                    nc.sync.dma_start(wih_sb[k][:], d_wiht.ap()[k * 128:(k + 1) * 128, :])
                nc.sync.dma_start(xt1_sb[:], d_xt.ap()[256:257, :])
                nc.sync.dma_start(wih1_sb[:], d_wiht.ap()[256:257, :])
                for m in range(32):            # (t,b) tiles of 128 rows
                    for nh in range(2):        # 1024 = 2 x 512
                        ps = pa_ps.tile([128, 512], f32, tag="xp_ps")
                        nsl = slice(nh * 512, (nh + 1) * 512)
                        msl = slice(m * 128, (m + 1) * 128)
                        nc.tensor.matmul(ps[:], xt_sb[0][:, msl], wih_sb[0][:, nsl],
                                         start=True, stop=False)
                        nc.tensor.matmul(ps[:], xt_sb[1][:, msl], wih_sb[1][:, nsl],
                                         start=False, stop=False)
                        nc.tensor.matmul(ps[:], xt1_sb[:, msl], wih1_sb[:, nsl],
                                         start=False, stop=True)
                        ot = pa_out.tile([128, 512], bf16, tag="xp_out")
                        nc.vector.tensor_copy(ot[:], ps[:])
                        nc.sync.dma_start(xp_ap[msl, nsl], ot[:])

            # ---------- phase B: ctx_proj^T = W_ctx @ ctx^T + (b_ctx+b_in) ------
            with tc.tile_pool(name="phb", bufs=1) as pb, \
                 tc.tile_pool(name="phb_ps", bufs=2, space="PSUM") as pb_ps:
                ctxt_sb = [pb.tile([128, BL * L], bf16, tag=f"ctxt{k}", name=f"ctxt{k}") for k in range(2)]
                wctx_sb = [pb.tile([128, H], bf16, tag=f"wctx{k}", name=f"wctx{k}") for k in range(2)]
                for k in range(2):
                    nc.sync.dma_start(ctxt_sb[k][:], d_ctxt.ap()[k * 128:(k + 1) * 128, :])
                    nc.sync.dma_start(wctx_sb[k][:], d_wctxt.ap()[k * 128:(k + 1) * 128, :])
                for m in range(2):             # h_out tile
                    for nchunk in range(8):    # BL*L = 4096 = 8 x 512
                        ps = pb_ps.tile([128, 512], f32, tag="cp_ps")
                        nsl = slice(nchunk * 512, (nchunk + 1) * 512)
                        for k in range(2):
                            nc.tensor.matmul(ps[:], wctx_sb[k][:, m * 128:(m + 1) * 128],
                                             ctxt_sb[k][:, nsl],
                                             start=(k == 0), stop=(k == 1))
                        nc.vector.tensor_scalar_add(cpT_sb[m][:, nsl], ps[:],
                                                    beta2_sb[m][:, 0:1])

            # ---------- main: recurrence + attention ----------
            with tc.tile_pool(name="gates_ps", bufs=2, space="PSUM") as gps_pool, \
                 tc.tile_pool(name="tr_ps", bufs=1, space="PSUM") as trp_pool, \
                 tc.tile_pool(name="ps512", bufs=3, space="PSUM") as ps512, \
                 tc.tile_pool(name="xp_in", bufs=6) as xp_pool, \
                 tc.tile_pool(name="hist", bufs=2) as hist_pool, \
                 tc.tile_pool(name="gtmp", bufs=2) as gtmp, \
                 tc.tile_pool(name="targ", bufs=2) as targ_pool, \
                 tc.tile_pool(name="tanh", bufs=4) as tanh_pool, \
                 tc.tile_pool(name="attev", bufs=3) as attev:

                hist_cur = None
                hist_prev_tiles = None

                def hist_k(tile_, k):
                    # hist layout: col = 16*(t%64) + 8*k + b
                    return tile_.rearrange("p (t k b) -> p t k b", k=2, b=BL)[:, :, k, :]

                def emit_inp_block(blk32, hist_tiles):
                    # inp^T cols [blk32*256 : +256] = W_in @ h_hist half-block
                    half = blk32 % 2
                    for m in range(2):
                        ps = ps512.tile([128, 512], f32, tag="att_ps", name="inp_ps")
                        for k in range(2):
                            rhs = hist_k(hist_tiles, k)[:, half * 32:(half + 1) * 32, :]
                            nc.tensor.matmul(ps[:, 0:256],
                                             wint_sb[k][:, m * 128:(m + 1) * 128],
                                             rhs,
                                             start=(k == 0), stop=(k == 1))
                        nc.vector.tensor_copy(
                            inpT_sb[m][:, blk32 * 256:(blk32 + 1) * 256], ps[:, 0:256])

                # ---- paced, dependency-aware attention emission ----
                # TB0/TB1: 4 waves of 2 b (1 b per psum bank).
                # TB2: 2 waves of 4 b (2 b per bank). TB3: 1 wave of 8 b (4 b/bank).
                att_state = {}

                def att_tile_geom(TB, tb8l, bs):
                    Nw = 512 - 128 * TB
                    jb = 8 * tb8l
                    offs = {}
                    off = 0
                    for b_idx in range(len(bs)):
                        for i in range(8):
                            c0 = (jb + i) & ~1
                            fd = Nw - c0
                            offs[(b_idx, i)] = (off, fd, c0)
                            off += fd
                    return offs, off

                def att_build_tiles(TB, tb8l, bs):
                    Nw = 512 - 128 * TB
                    l0 = 128 * TB
                    st = att_state
                    offs, total = att_tile_geom(TB, tb8l, bs)
                    st["tanh"] = {}
                    st["geom"] = offs
                    for k in range(2):
                        tg = targ_pool.tile([128, 8200], bf16, tag="targ",
                                            name="targ")
                        for b_idx, b in enumerate(bs):
                            for i in range(8):
                                t_g = l0 + 8 * tb8l + i
                                off, fd, c0 = offs[(b_idx, i)]
                                nc.vector.tensor_scalar_add(
                                    tg[:, off:off + fd],
                                    cpT_sb[k][:, b * 512 + l0 + c0:(b + 1) * 512],
                                    inpT_sb[k][:, t_g * 8 + b:t_g * 8 + b + 1])
                        th = tanh_pool.tile([128, 8200], bf16, tag="tanh",
                                            name="tanh")
                        nc.scalar.activation(th[:, 0:total], tg[:, 0:total],
                                             AFT.Tanh)
                        st["tanh"][k] = th

                def att_evac(TB, bs, nbank):
                    Nw = 512 - 128 * TB
                    l0 = 128 * TB
                    st = att_state
                    per = len(bs) // nbank
                    for b_idx, b in enumerate(bs):
                        ps = st["ps"][b_idx // per]
                        boff = (b_idx % per) * Nw
                        ev = attev.tile([128, 512], f32, tag="attev", name="attev")
                        nc.vector.tensor_add(ev[:, 0:Nw], ps[:, boff:boff + Nw],
                                             mtri_sb[:, 0:Nw])
                        nc.sync.dma_start(
                            att_ap[b, TB * 128:(TB + 1) * 128, l0:512], ev[:, 0:Nw])
                        if TB > 0:
                            nc.sync.dma_start(
                                att_ap[b, TB * 128:(TB + 1) * 128, 0:l0],
                                inf_sb[:, 0:l0])
                    st.clear()

                def emit_att_unit(TB, wave, unit, bs, nbank, njpu):
                    Nw = 512 - 128 * TB
                    st = att_state
                    per = len(bs) // nbank
                    if unit == 0:
                        st["ps"] = [ps512.tile([128, 512], f32, tag="att_ps",
                                               name="att_ps") for _ in range(nbank)]
                        if per > 1:
                            for ps in st["ps"]:
                                nc.tensor.matmul(ps[:], vwin_sb[0][:, 0:128],
                                                 cpT_sb[0][:, 0:512],
                                                 start=True, stop=False,
                                                 skip_group_check=True)
                    j0 = njpu * unit
                    if (j0 % 8) == 0:
                        att_build_tiles(TB, j0 // 8, bs)
                    offs = st["geom"]
                    for j in range(j0, j0 + njpu):
                        i = j % 8
                        for k in range(2):
                            lw = vwin_sb[k][:, 128 - j:256 - j]
                            for b_idx in range(len(bs)):
                                off, fd, c0 = offs[(b_idx, i)]
                                ps = st["ps"][b_idx // per]
                                boff = (b_idx % per) * Nw
                                nc.tensor.matmul(
                                    ps[:, boff + c0:boff + Nw],
                                    lw, st["tanh"][k][:, off:off + fd],
                                    start=(per == 1 and j == 0 and k == 0),
                                    stop=(j == 127 and k == 1 and
                                          b_idx == len(bs) - 1),
                                    skip_group_check=True)
                    if j0 + njpu == 128:
                        att_evac(TB, bs, nbank)

                slices = []
                for TB in range(2):
                    for wave in range(4):
                        bs2 = (2 * wave, 2 * wave + 1)
                        for unit in range(32):
                            need = 128 * TB + (8 * (unit // 2) + 7 if unit % 2 == 0
                                               else 4 * unit + 3)
                            slices.append((TB, wave, unit, bs2, 2, 4, need))
                for wave in range(2):
                    bs4 = tuple(range(4 * wave, 4 * wave + 4))
                    for unit in range(64):
                        need = 256 + (8 * (unit // 4) + 7 if unit % 4 == 0
                                      else 2 * unit + 1)
                        slices.append((2, wave, unit, bs4, 2, 2, need))
                bs8 = tuple(range(8))
                for unit in range(64):
                    need = 384 + (8 * (unit // 4) + 7 if unit % 4 == 0
                                  else 2 * unit + 1)
                    slices.append((3, 0, unit, bs8, 2, 2, need))
                # cost-weighted scheduled emission step (ses) per unit
                def unit_cost(TB, unit, bs, njpu):
                    Nw = 512 - 128 * TB
                    return sum(2 * len(bs) * (Nw - ((njpu * unit + jj) & ~1))
                               for jj in range(njpu))
                costs = [unit_cost(s[0], s[2], s[3], s[5]) for s in slices]
                csum = 0.0
                total_c = float(sum(costs))
                T0, T1 = 156, L
                ses = []
                for c in costs:
                    ses.append(T0 + int(csum / total_c * (T1 - T0)))
                    csum += c
                sl_i = 0
                TOT = len(slices)

                for t in range(L):
                    if t % 64 == 0:
                        hist_prev_tiles = hist_cur
                        hist_cur = hist_pool.tile([128, 64 * 16], bf16, tag="hist",
                                                  name="hist")
                    # ---- gates = x_proj[t] + h @ W_hh.T  (PSUM accumulate) ----
                    gps = gps_pool.tile([BL, H4], f32, tag="gates")
                    xpt = xp_pool.tile([BL, H4], bf16, tag="xp")
                    nc.sync.dma_start(xpt[:], xp_ap[t * BL:(t + 1) * BL, :])
                    for nh in range(2):
                        nsl = slice(nh * 512, (nh + 1) * 512)
                        nc.tensor.matmul(gps[:, nsl], i8b_sb[:], xpt[:, nsl],
                                         start=True, stop=False)
                        for k in range(2):
                            if t == 0:
                                lhsT = h0t_sb[k][:]
                            else:
                                src = hist_cur if (t % 64) != 0 else hist_prev_tiles
                                u = (t - 1) % 64
                                lhsT = src[:, 16 * u + 8 * k:16 * u + 8 * k + 8]
                            nc.tensor.matmul(gps[:, nsl], lhsT, whht_sb[k][:, nsl],
                                             start=False, stop=(k == 1))
                    # ---- nonlinearities (bf16 temps) ----
                    sifo = gtmp.tile([BL, 768], bf16, tag="sifo")
                    tg_t = gtmp.tile([BL, 256], bf16, tag="tg")
                    nc.scalar.activation(sifo[:, 0:512], gps[:, 0:512], AFT.Sigmoid)
                    nc.scalar.activation(tg_t[:], gps[:, 768:1024], AFT.Tanh)
                    nc.scalar.activation(sifo[:, 512:768], gps[:, 512:768],
                                         AFT.Sigmoid)
                    # ---- c update ----
                    t1 = gtmp.tile([BL, 256], bf16, tag="t1")
                    t2 = gtmp.tile([BL, 256], bf16, tag="t2")
                    nc.vector.tensor_mul(t2[:], sifo[:, 0:256], tg_t[:])
                    nc.vector.tensor_mul(t1[:], sifo[:, 256:512], c_sb[:])
                    nc.vector.tensor_add(c_sb[:], t1[:], t2[:])
                    tc_t = gtmp.tile([BL, 256], bf16, tag="tc")
                    nc.scalar.activation(tc_t[:], c_sb[:], AFT.Tanh)
                    nc.vector.tensor_mul(h_sb[:], sifo[:, 512:768], tc_t[:])
                    # ---- h^T into history (PE transpose, one fused copy) ----
                    trp = trp_pool.tile([128, 16], bf16, tag="tr")
                    nc.tensor.transpose(trp[:, 0:8], h_sb[:, 0:128], i8b_sb[:])
                    nc.tensor.transpose(trp[:, 8:16], h_sb[:, 128:256], i8b_sb[:])
                    nc.vector.tensor_copy(hist_cur[:, 16 * (t % 64):16 * (t % 64) + 16],
                                          trp[:])

                    if t % 32 == 31:
                        emit_inp_block(t // 32, hist_cur)
                    # paced attention emission
                    avail = t if t % 32 == 31 else (t // 32) * 32 - 1
                    nem = 0
                    while (sl_i < TOT and nem < 3 and ses[sl_i] <= t
                           and slices[sl_i][-1] <= avail):
                        s = slices[sl_i]
                        emit_att_unit(*s[:6])
                        sl_i += 1
                        nem += 1

                while sl_i < TOT:
                    s = slices[sl_i]
                    emit_att_unit(*s[:6])
                    sl_i += 1

                hf = gtmp.tile([BL, 256], f32, tag="hf")
                cf = gtmp.tile([BL, 256], f32, tag="cf")
                nc.vector.tensor_copy(hf[:], h_sb[:])
                nc.vector.tensor_copy(cf[:], c_sb[:])
                nc.sync.dma_start(d_hT.ap()[:], hf[:])
                nc.sync.dma_start(d_cT.ap()[:], cf[:])

    nc.compile()
    return nc


def _get_program():
    global _PROG
    if _PROG is None:
        _PROG = _build_program()
    return _PROG


def _prep_core_inputs(inputs, core):
    """Host-side layout prep for one core's batch shard (glue only)."""
    bf16 = ml_dtypes.bfloat16
    bsl = slice(core * BL, (core + 1) * BL)
    emb = np.asarray(inputs["embedded_inputs"])[bsl]    # [BL, L, E]
    dec = np.asarray(inputs["decoder_input"])[bsl]      # [BL, E]
    h0 = np.asarray(inputs["h0"])[bsl]
    c0 = np.asarray(inputs["c0"])[bsl]
    ctx = np.asarray(inputs["context"])[bsl]            # [BL, L, H]

    gate_perm = np.r_[0:256, 256:512, 768:1024, 512:768]  # [i,f,g,o] -> [i,f,o,g]
    W_ih = np.asarray(inputs["W_ih"])[gate_perm]
    W_hh = np.asarray(inputs["W_hh"])[gate_perm]
    beta_g = (np.asarray(inputs["b_ih"]) + np.asarray(inputs["b_hh"]))[gate_perm]
    W_in = np.asarray(inputs["W_in"])
    W_ctx = np.asarray(inputs["W_ctx"])
    beta2 = (np.asarray(inputs["b_in"]) + np.asarray(inputs["b_ctx"]))
    V = np.asarray(inputs["V"])

    # X[t] = dec if t==0 else emb[:, t-1];  xt = [X^T; ones], cols (t, b)
    X = np.concatenate([dec[:, None, :], emb[:, :L - 1, :]], axis=1)  # [BL, L, E]
    xt = np.empty((257, L * BL), dtype=bf16)
    xt[:256] = X.transpose(2, 1, 0).reshape(E, L * BL).astype(bf16)
    xt[256] = np.ones(L * BL, dtype=bf16)

    ctxt = ctx.transpose(2, 0, 1).reshape(H, BL * L).astype(bf16)     # (h, (b,l))

    wiht = np.empty((257, H4), dtype=bf16)
    wiht[:256] = W_ih.T.astype(bf16)
    wiht[256] = beta_g.astype(bf16)

    vwin = np.zeros((256, 257), dtype=bf16)
    vwin[:, 128] = V.astype(bf16)

    mtri = np.zeros((128, 512), dtype=np.float32)
    r = np.arange(128)
    mask = np.arange(512)[None, :] < r[:, None]
    mtri[mask] = NEG_INF

    return {
        "xt": xt,
        "ctxt": ctxt,
        "wiht": wiht,
        "whht": np.ascontiguousarray(W_hh.T).astype(bf16),
        "wint": np.ascontiguousarray(W_in.T).astype(bf16),
        "wctxt": np.ascontiguousarray(W_ctx.T).astype(bf16),
        "beta2": beta2.astype(np.float32).reshape(256, 1),
        "vwin": vwin,
        "mtri": mtri,
        "i8f": np.eye(8, dtype=np.float32),
        "i8b": np.eye(8, dtype=bf16),
        "h0t": np.ascontiguousarray(h0.T).astype(bf16),
        "c0": np.ascontiguousarray(c0).astype(bf16),
    }


LAST_RESULT = None


def kernel(**inputs):
    from concourse.bass_utils import run_bass_kernel_spmd

    nc = _get_program()
    in_maps = [_prep_core_inputs(inputs, c) for c in range(NCORES)]
    res = run_bass_kernel_spmd(nc, in_maps, list(range(NCORES)))
    global LAST_RESULT
    LAST_RESULT = res

    outputs = np.empty((B, L, L), dtype=np.float32)
    hT = np.empty((B, H), dtype=np.float32)
    cT = np.empty((B, H), dtype=np.float32)
    for c in range(NCORES):
        bsl = slice(c * BL, (c + 1) * BL)
        outputs[bsl] = res.results[c]["att"]
        hT[bsl] = res.results[c]["hT_out"]
        cT[bsl] = res.results[c]["cT_out"]
    pointers = np.broadcast_to(np.arange(L, dtype=np.int32), (B, L)).copy()
    return outputs, pointers, hT, cT
